# revision 5
# baseline (speedup 1.0000x reference)
"""Contrastive (NT-Xent-style) loss kernel for Trainium2, 8 NeuronCores.

Problem: z1, z2 [16384, 256] fp32.
  h1 = l2norm(z1, axis=1); h2 = l2norm(z2, axis=1)
  sim = h1 @ h2.T                       [N, N]
  between = exp(sim / tau)
  loss = sum_i -log(diag_i / (rowsum_i - diag_i))
       = sum_i [ log(rowsum_i - diag_i) - sim_ii / tau ]

Sharding: z1 rows split across 8 cores (2048 rows each); z2 replicated.
Each core streams its [2048, 16384] similarity block through PSUM in
[128, 2048] tiles, applies exp on the scalar (ACT) engine with fused
row-accumulation (accum_out), and only row-sums + the diagonal ever
materialize.  Per-core output is [128, 16] per-row loss terms; the host
sums them (the "all-reduce" of the scalar loss).

Matmul runs in bf16 (inputs normalized in fp32 then rounded); everything
else (norms, exp accumulation, log) is fp32.
"""

import numpy as np

# ---- problem constants (hardcoded per contract) ----
N_FULL = 16384
D = 256
TAU = 0.2
N_CORES = 8
P = 128                      # partitions
M_LOC = N_FULL // N_CORES    # 2048 z1 rows per core
M_TILES = M_LOC // P         # 16
G = 8                        # z2 row groups per core
G_ROWS = N_FULL // G         # 2048 z2 rows per group
G_TILES = G_ROWS // P        # 16
NSUB = 4                     # 512-wide matmul sub-chunks per psum tile
PSUM_N = NSUB * 512          # 2048
KD = 2                       # contraction split: 256 = 2 x 128

_CACHE = {}


def _patch_tile_drain():
    """This walrus build allows at most one sem-wait per instruction, but
    TileContext's exit path parks every global-clock wait on a single Drain
    ("Too many sync wait commands" at codegen).  Respread the waits across
    single-wait nops on the sync queue, all still ahead of the barrier."""
    import concourse.tile as tile
    from concourse.vector_clock import ScopedClock

    if getattr(tile.TileContext, "_drain_waits_patched", False):
        return

    def _drain_and_barrier(self, tick_clock, wait_clock):
        nc = self.nc
        drain_inst = nc.sync.drain()
        wait_clock.add_sem_waits(
            drain_inst.ins, ScopedClock({None: tick_clock.global_clock})
        )
        si = drain_inst.ins.sync_info
        waits = list(si.on_wait) if si is not None and si.on_wait else []
        if len(waits) > 1:
            drain_inst.ins.sync_info = None
            assert self.sems is not None
            id2handle = {h.num: h for h in self.sems.allocated().values()}
            for w in waits:
                h = id2handle[w.id]
                nc.sync.nop(nofuse=True).wait_op(h, w.wait_value, "sem-ge")
        nc.all_engine_barrier()
        assert self.sems is not None
        popped = nc._tile_sem_poison_stack.pop()
        assert popped is self._sem_poison
        nc.clear_and_free_semaphores(list(self.sems.allocated().values()))
        nc.all_engine_barrier()

    tile.TileContext._drain_and_barrier = _drain_and_barrier
    tile.TileContext._drain_waits_patched = True


def _build_nc():
    from contextlib import ExitStack

    import concourse.bacc as bacc
    import concourse.tile as tile
    from concourse import mybir
    from concourse.masks import make_identity

    AF = mybir.ActivationFunctionType
    ALU = mybir.AluOpType
    FP32 = mybir.dt.float32
    BF16 = mybir.dt.bfloat16

    # Bacc (not raw Bass): its compile() pass legalizes multi-wait
    # instructions into event semaphores — this walrus build rejects >1
    # sem-wait per instruction ("Too many sync wait commands").
    nc = bacc.Bacc("TRN2", target_bir_lowering=False, debug=False)

    z1 = nc.dram_tensor("z1", [M_LOC, D], FP32, kind="ExternalInput").ap()
    z2 = nc.dram_tensor("z2", [N_FULL, D], FP32, kind="ExternalInput").ap()
    z2d = nc.dram_tensor("z2d", [M_LOC, D], FP32, kind="ExternalInput").ap()
    out_parts = nc.dram_tensor(
        "loss_parts", [P, M_TILES], FP32, kind="ExternalOutput"
    ).ap()

    with tile.TileContext(nc) as tc, ExitStack() as ctx:
        pz1 = ctx.enter_context(tc.tile_pool(name="z1p", bufs=1))
        pz2d = ctx.enter_context(tc.tile_pool(name="z2dp", bufs=1))
        pzg = ctx.enter_context(tc.tile_pool(name="zgp", bufs=2))
        ph1 = ctx.enter_context(tc.tile_pool(name="h1p", bufs=1))
        ph2 = ctx.enter_context(tc.tile_pool(name="h2p", bufs=2))
        pid = ctx.enter_context(tc.tile_pool(name="idp", bufs=1))
        pscr = ctx.enter_context(tc.tile_pool(name="scrp", bufs=4))
        phbf = ctx.enter_context(tc.tile_pool(name="hbfp", bufs=4))
        pesc = ctx.enter_context(tc.tile_pool(name="escp", bufs=2))
        pst = ctx.enter_context(tc.tile_pool(name="stats", bufs=1))
        pgst = ctx.enter_context(tc.tile_pool(name="gstats", bufs=2))
        ppsum = ctx.enter_context(tc.tile_pool(name="psump", bufs=2, space="PSUM"))

        ident = pid.tile([P, P], BF16, tag="ident")
        make_identity(nc, ident[:])

        def sumsq(dst, a, b):
            """dst[:,1] = sum over free dim of a*b (DVE, one op)."""
            s = pscr.tile([P, D], BF16, tag="scr")
            nc.vector.scalar_tensor_tensor(
                s[:], in0=a, scalar=1.0, in1=b,
                op0=ALU.mult, op1=ALU.mult, accum_out=dst,
            )

        def rnorm_of(ssq, pool, tag):
            """exp(-0.5*ln(ssq)) == 1/sqrt(ssq); stays in the exp/ln ACT set
            (Rsqrt on ACT is banned for accuracy)."""
            ln = pool.tile([P, ssq.shape[1]], FP32, tag=tag + "_ln")
            nc.scalar.activation(ln[:], ssq, AF.Ln)
            rn = pool.tile([P, ssq.shape[1]], FP32, tag=tag)
            nc.scalar.activation(rn[:], ln[:], AF.Exp, scale=-0.5)
            return rn

        def build_transposed(dst, src_tiles, rn, n_tiles):
            """dst [P, KD, n_tiles*P] bf16 <- per-tile normalize + PE transpose.

            src_tiles[t] is the raw fp32 [P, D] tile; rn [P, n_tiles] the
            per-row 1/norm.  Transposes batch through two [P, n_tiles, P]
            bf16 PSUM tiles (shared slot with the matmul psum pool), then one
            DVE copy per contraction half moves them into SBUF."""
            hbs = []
            for t in range(n_tiles):
                hb = phbf.tile([P, D], BF16, tag="hbf")
                nc.vector.tensor_scalar(
                    hb[:], src_tiles(t), rn[:, t : t + 1], None, ALU.mult
                )
                hbs.append(hb)
            for kk in range(KD):
                pt = ppsum.tile([P, n_tiles, P], BF16, tag="ps")
                for t in range(n_tiles):
                    nc.tensor.transpose(
                        pt[:, t, :], hbs[t][:, kk * P : (kk + 1) * P], ident[:]
                    )
                nc.vector.tensor_copy(dst[:, kk, :], pt[:, :, :])

        # ---------- z1 / z2d prep ----------
        z1t = pz1.tile([P, M_TILES, D], FP32, tag="z1t")
        nc.scalar.dma_start(z1t[:], z1.rearrange("(t p) d -> p t d", p=P))
        z2dt = pz2d.tile([P, M_TILES, D], FP32, tag="z2dt")
        nc.scalar.dma_start(z2dt[:], z2d.rearrange("(t p) d -> p t d", p=P))

        ssq1 = pst.tile([P, M_TILES], FP32, tag="ssq1")
        ssq2d = pst.tile([P, M_TILES], FP32, tag="ssq2d")
        d_raw = pst.tile([P, M_TILES], FP32, tag="d_raw")
        for m in range(M_TILES):
            a = z1t[:, m, :]
            b = z2dt[:, m, :]
            sumsq(ssq1[:, m : m + 1], a, a)
            sumsq(ssq2d[:, m : m + 1], b, b)
            sumsq(d_raw[:, m : m + 1], a, b)

        rn1 = rnorm_of(ssq1[:], pst, "rn1")
        rn2d = rnorm_of(ssq2d[:], pst, "rn2d")

        h1T = ph1.tile([P, KD, M_LOC], BF16, tag="h1T")
        build_transposed(h1T, lambda t: z1t[:, t, :], rn1, M_TILES)

        parts = pst.tile([P, M_TILES * G], FP32, tag="parts")

        # ---------- main loop over z2 groups ----------
        for g in range(G):
            zgt = pzg.tile([P, G_TILES, D], FP32, tag="zgt")
            nc.scalar.dma_start(
                zgt[:],
                z2[g * G_ROWS : (g + 1) * G_ROWS, :].rearrange(
                    "(t p) d -> p t d", p=P
                ),
            )
            ssqg = pgst.tile([P, G_TILES], FP32, tag="ssqg")
            for t in range(G_TILES):
                sumsq(ssqg[:, t : t + 1], zgt[:, t, :], zgt[:, t, :])
            rng = rnorm_of(ssqg[:], pgst, "rng")

            h2T = ph2.tile([P, KD, G_ROWS], BF16, tag="h2T")
            build_transposed(h2T, lambda t: zgt[:, t, :], rng, G_TILES)

            for m in range(M_TILES):
                ps = ppsum.tile([P, PSUM_N], FP32, tag="ps")
                for k in range(KD):
                    for sub in range(NSUB):
                        nc.tensor.matmul(
                            ps[:, sub * 512 : (sub + 1) * 512],
                            h1T[:, k, m * P : (m + 1) * P],
                            h2T[:, k, sub * 512 : (sub + 1) * 512],
                            start=(k == 0),
                            stop=(k == KD - 1),
                        )
                esc = pesc.tile([P, PSUM_N], BF16, tag="esc")
                nc.scalar.activation(
                    esc[:], ps[:], AF.Exp, scale=1.0 / TAU,
                    accum_out=parts[:, m * G + g : m * G + g + 1],
                )

        # ---------- finalize ----------
        st = pst.tile([P, M_TILES], FP32, tag="st")
        nc.vector.tensor_mul(st[:], d_raw[:], rn1[:])
        nc.vector.tensor_mul(st[:], st[:], rn2d[:])
        nc.vector.tensor_scalar(st[:], st[:], 1.0 / TAU, None, ALU.mult)
        dex = pst.tile([P, M_TILES], FP32, tag="dex")
        nc.scalar.activation(dex[:], st[:], AF.Exp)
        rows = pst.tile([P, M_TILES], FP32, tag="rows")
        nc.vector.tensor_reduce(
            rows[:],
            parts[:].rearrange("p (m g) -> p m g", g=G),
            axis=mybir.AxisListType.X,
            op=ALU.add,
        )
        neg = pst.tile([P, M_TILES], FP32, tag="neg")
        nc.vector.tensor_sub(neg[:], rows[:], dex[:])
        lneg = pst.tile([P, M_TILES], FP32, tag="lneg")
        nc.scalar.activation(lneg[:], neg[:], AF.Ln)
        lp = pst.tile([P, M_TILES], FP32, tag="lp")
        nc.vector.tensor_sub(lp[:], lneg[:], st[:])
        nc.sync.dma_start(out_parts, lp[:])

    nc.compile()
    return nc


def get_nc():
    if "nc" not in _CACHE:
        _CACHE["nc"] = _build_nc()
    return _CACHE["nc"]


def make_in_maps(z1, z2):
    z1 = np.ascontiguousarray(np.asarray(z1, dtype=np.float32))
    z2 = np.ascontiguousarray(np.asarray(z2, dtype=np.float32))
    in_maps = []
    for c in range(N_CORES):
        blk = slice(c * M_LOC, (c + 1) * M_LOC)
        in_maps.append({"z1": z1[blk], "z2": z2, "z2d": z2[blk]})
    return in_maps


def kernel(z1, z2):
    from concourse.bass_utils import run_bass_kernel_spmd

    nc = get_nc()
    res = run_bass_kernel_spmd(nc, make_in_maps(z1, z2), core_ids=list(range(N_CORES)))
    total = 0.0
    for c in range(N_CORES):
        total += res.results[c]["loss_parts"].astype(np.float64).sum()
    return np.float32(total)


# revision 8
# speedup vs baseline: 1.0280x; 1.0280x over previous
"""Contrastive (NT-Xent-style) loss kernel for Trainium2, 8 NeuronCores.

Problem: z1, z2 [16384, 256] fp32.
  h1 = l2norm(z1, axis=1); h2 = l2norm(z2, axis=1)
  sim = h1 @ h2.T                       [N, N]
  between = exp(sim / tau)
  loss = sum_i -log(diag_i / (rowsum_i - diag_i))
       = sum_i [ log(rowsum_i - diag_i) - sim_ii / tau ]

Sharding: z1 rows split across 8 cores (2048 rows each); z2 replicated.
Each core streams its [2048, 16384] similarity block through PSUM in
[128, 2048] tiles, applies exp on the scalar (ACT) engine with fused
row-accumulation (accum_out), and only row-sums + the diagonal ever
materialize.  Per-core output is [128, 16] per-row loss terms; the host
sums them (the "all-reduce" of the scalar loss).

Matmul runs in bf16 (inputs normalized in fp32 then rounded); everything
else (norms, exp accumulation, log) is fp32.  1/||row|| is a DVE-only
Newton rsqrt (bit-trick seed) so the ACT engine never leaves the exp
table set mid-kernel.  z2-group transposes for group g+1 are emitted in
small bursts between group g's matmuls to keep the PE HAM clock warm.
"""

import numpy as np

# ---- problem constants (hardcoded per contract) ----
N_FULL = 16384
D = 256
TAU = 0.2
N_CORES = 8
P = 128                      # partitions
M_LOC = N_FULL // N_CORES    # 2048 z1 rows per core
M_TILES = M_LOC // P         # 16
G = 8                        # z2 row groups per core
G_ROWS = N_FULL // G         # 2048 z2 rows per group
G_TILES = G_ROWS // P        # 16
NSUB = 4                     # 512-wide matmul sub-chunks per psum tile
PSUM_N = NSUB * 512          # 2048
KD = 2                       # contraction split: 256 = 2 x 128
RSQRT_MAGIC = 0x5F3759DF

_CACHE = {}


def _build_nc():
    from contextlib import ExitStack

    import concourse.bacc as bacc
    import concourse.tile as tile
    from concourse import mybir
    from concourse.masks import make_identity

    AF = mybir.ActivationFunctionType
    ALU = mybir.AluOpType
    FP32 = mybir.dt.float32
    INT32 = mybir.dt.int32
    BF16 = mybir.dt.bfloat16

    # Bacc (not raw Bass): its compile() pass legalizes multi-wait
    # instructions into event semaphores — this walrus build rejects >1
    # sem-wait per instruction ("Too many sync wait commands").
    nc = bacc.Bacc("TRN2", target_bir_lowering=False, debug=False)

    z1 = nc.dram_tensor("z1", [M_LOC, D], FP32, kind="ExternalInput").ap()
    z2 = nc.dram_tensor("z2", [N_FULL, D], FP32, kind="ExternalInput").ap()
    z2d = nc.dram_tensor("z2d", [M_LOC, D], FP32, kind="ExternalInput").ap()
    out_parts = nc.dram_tensor(
        "loss_parts", [P, M_TILES], FP32, kind="ExternalOutput"
    ).ap()

    with tile.TileContext(nc) as tc, ExitStack() as ctx:
        pz1 = ctx.enter_context(tc.tile_pool(name="z1p", bufs=1))
        pz2d = ctx.enter_context(tc.tile_pool(name="z2dp", bufs=1))
        pzg = ctx.enter_context(tc.tile_pool(name="zgp", bufs=2))
        ph1 = ctx.enter_context(tc.tile_pool(name="h1p", bufs=1))
        ph2 = ctx.enter_context(tc.tile_pool(name="h2p", bufs=2))
        pid = ctx.enter_context(tc.tile_pool(name="idp", bufs=1))
        pscr = ctx.enter_context(tc.tile_pool(name="scrp", bufs=4))
        phbf = ctx.enter_context(tc.tile_pool(name="hbfp", bufs=12))
        pesc = ctx.enter_context(tc.tile_pool(name="escp", bufs=2))
        pst = ctx.enter_context(tc.tile_pool(name="stats", bufs=1))
        pgst = ctx.enter_context(tc.tile_pool(name="gstats", bufs=2))
        ppsum = ctx.enter_context(tc.tile_pool(name="psump", bufs=2, space="PSUM"))

        ident = pid.tile([P, P], BF16, tag="ident")
        make_identity(nc, ident[:])

        def sumsq(dst, a, b):
            """dst[:,1] = sum over free dim of a*b (DVE, one op)."""
            s = pscr.tile([P, D], BF16, tag="scr")
            nc.vector.scalar_tensor_tensor(
                s[:], in0=a, scalar=1.0, in1=b,
                op0=ALU.mult, op1=ALU.mult, accum_out=dst,
            )

        def rsqrt_dve(ssq, pool, tag, w):
            """1/sqrt(ssq) entirely on DVE: bit-trick seed + 2 Newton steps.
            Keeps ACT parked in the exp table set (Ln would thrash it)."""
            y = pool.tile([P, w], FP32, tag=tag)
            t1 = pool.tile([P, w], FP32, tag=tag + "_t1")
            t2 = pool.tile([P, w], FP32, tag=tag + "_t2")
            yi = y[:].bitcast(INT32)
            # yi = MAGIC - (u >> 1); shift (bitwise) and mult/add (arith)
            # must be separate instructions — walrus rejects mixed-class ops.
            nc.vector.tensor_scalar(
                yi, ssq.bitcast(INT32), 1, None, ALU.logical_shift_right
            )
            nc.vector.tensor_scalar(yi, yi, -1, RSQRT_MAGIC, ALU.mult, ALU.add)
            for _ in range(2):
                # y *= 1.5 - 0.5*ssq*y*y
                nc.vector.tensor_mul(t1[:], y[:], y[:])
                nc.vector.scalar_tensor_tensor(
                    t2[:], in0=ssq, scalar=-0.5, in1=t1[:],
                    op0=ALU.mult, op1=ALU.mult,
                )
                nc.vector.tensor_scalar(t2[:], t2[:], 1.5, None, ALU.add)
                nc.vector.tensor_mul(y[:], y[:], t2[:])
            return y

        def norm_tiles(zt, rn, t0, nt):
            """Normalized bf16 [P, D] tiles for rows t0..t0+nt-1."""
            hbs = []
            for t in range(t0, t0 + nt):
                hb = phbf.tile([P, D], BF16, tag="hbf")
                nc.vector.tensor_scalar(
                    hb[:], zt[:, t, :], rn[:, t : t + 1], None, ALU.mult
                )
                hbs.append(hb)
            return hbs

        def xpose_burst(hbs, kk, dst, t0):
            """PE-transpose one contraction half of len(hbs) tiles into dst."""
            n = len(hbs)
            pt = ppsum.tile([P, n, P], BF16, tag="ps")
            for j, hb in enumerate(hbs):
                nc.tensor.transpose(
                    pt[:, j, :], hb[:, kk * P : (kk + 1) * P], ident[:]
                )
            nc.vector.tensor_copy(dst[:, kk, t0 * P : (t0 + n) * P], pt[:, :, :])

        # ---------- prologue: z1 / z2d / group-0 prep ----------
        z1t = pz1.tile([P, M_TILES, D], FP32, tag="z1t")
        nc.sync.dma_start(z1t[:], z1.rearrange("(t p) d -> p t d", p=P))
        z2dt = pz2d.tile([P, M_TILES, D], FP32, tag="z2dt")
        nc.sync.dma_start(z2dt[:], z2d.rearrange("(t p) d -> p t d", p=P))

        def load_group(g):
            zt = pzg.tile([P, G_TILES, D], FP32, tag="zgt")
            nc.sync.dma_start(
                zt[:],
                z2[g * G_ROWS : (g + 1) * G_ROWS, :].rearrange(
                    "(t p) d -> p t d", p=P
                ),
            )
            return zt

        ssq1 = pst.tile([P, M_TILES], FP32, tag="ssq1")
        ssq2d = pst.tile([P, M_TILES], FP32, tag="ssq2d")
        d_raw = pst.tile([P, M_TILES], FP32, tag="d_raw")
        for m in range(M_TILES):
            sumsq(ssq1[:, m : m + 1], z1t[:, m, :], z1t[:, m, :])
        rn1 = rsqrt_dve(ssq1[:], pst, "rn1", M_TILES)

        h1T = ph1.tile([P, KD, M_LOC], BF16, tag="h1T")
        hb1 = norm_tiles(z1t, rn1, 0, M_TILES)
        for kk in range(KD):
            xpose_burst(hb1[:8], kk, h1T, 0)
            xpose_burst(hb1[8:], kk, h1T, 8)

        # group 0 prep (batched; PE is cold here regardless)
        zgt_cur = load_group(0)
        ssqg = pgst.tile([P, G_TILES], FP32, tag="ssqg")
        for t in range(G_TILES):
            sumsq(ssqg[:, t : t + 1], zgt_cur[:, t, :], zgt_cur[:, t, :])
        rng = rsqrt_dve(ssqg[:], pgst, "rng", G_TILES)
        h2T_cur = ph2.tile([P, KD, G_ROWS], BF16, tag="h2T")
        hbs = norm_tiles(zgt_cur, rng, 0, G_TILES)
        for kk in range(KD):
            xpose_burst(hbs[:8], kk, h2T_cur, 0)
            xpose_burst(hbs[8:], kk, h2T_cur, 8)

        for m in range(M_TILES):
            sumsq(ssq2d[:, m : m + 1], z2dt[:, m, :], z2dt[:, m, :])
            sumsq(d_raw[:, m : m + 1], z1t[:, m, :], z2dt[:, m, :])
        rn2d = rsqrt_dve(ssq2d[:], pst, "rn2d", M_TILES)

        parts = pst.tile([P, M_TILES * G], FP32, tag="parts")

        # ---------- main loop over z2 groups ----------
        # Group g+1's load/norms/transposes are emitted in bursts between
        # group g's matmuls: the PE never sits idle long enough for HAM to
        # re-throttle, and prep fully hides under the exp stream.
        for g in range(G):
            nxt = {}
            for m in range(M_TILES):
                ps = ppsum.tile([P, PSUM_N], FP32, tag="ps")
                for k in range(KD):
                    for sub in range(NSUB):
                        nc.tensor.matmul(
                            ps[:, sub * 512 : (sub + 1) * 512],
                            h1T[:, k, m * P : (m + 1) * P],
                            h2T_cur[:, k, sub * 512 : (sub + 1) * 512],
                            start=(k == 0),
                            stop=(k == KD - 1),
                        )
                esc = pesc.tile([P, PSUM_N], BF16, tag="esc")
                nc.scalar.activation(
                    esc[:], ps[:], AF.Exp, scale=1.0 / TAU,
                    accum_out=parts[:, m * G + g : m * G + g + 1],
                )
                if g + 1 < G:
                    if m == 0:
                        nxt["zt"] = load_group(g + 1)
                        ssqn = pgst.tile([P, G_TILES], FP32, tag="ssqg")
                        for t in range(G_TILES):
                            sumsq(
                                ssqn[:, t : t + 1],
                                nxt["zt"][:, t, :],
                                nxt["zt"][:, t, :],
                            )
                        nxt["rn"] = rsqrt_dve(ssqn[:], pgst, "rng", G_TILES)
                        h2T_nxt = ph2.tile([P, KD, G_ROWS], BF16, tag="h2T")
                        nxt["h2T"] = h2T_nxt
                    elif m == 3:
                        nxt["hb_lo"] = norm_tiles(nxt["zt"], nxt["rn"], 0, 8)
                        xpose_burst(nxt["hb_lo"], 0, nxt["h2T"], 0)
                    elif m == 7:
                        xpose_burst(nxt["hb_lo"], 1, nxt["h2T"], 0)
                    elif m == 11:
                        nxt["hb_hi"] = norm_tiles(nxt["zt"], nxt["rn"], 8, 8)
                        xpose_burst(nxt["hb_hi"], 0, nxt["h2T"], 8)
                    elif m == 15:
                        xpose_burst(nxt["hb_hi"], 1, nxt["h2T"], 8)
            if g + 1 < G:
                zgt_cur = nxt["zt"]
                h2T_cur = nxt["h2T"]

        # ---------- finalize ----------
        st = pst.tile([P, M_TILES], FP32, tag="st")
        nc.vector.tensor_mul(st[:], d_raw[:], rn1[:])
        nc.vector.tensor_mul(st[:], st[:], rn2d[:])
        nc.vector.tensor_scalar(st[:], st[:], 1.0 / TAU, None, ALU.mult)
        dex = pst.tile([P, M_TILES], FP32, tag="dex")
        nc.scalar.activation(dex[:], st[:], AF.Exp)
        rows = pst.tile([P, M_TILES], FP32, tag="rows")
        nc.vector.tensor_reduce(
            rows[:],
            parts[:].rearrange("p (m g) -> p m g", g=G),
            axis=mybir.AxisListType.X,
            op=ALU.add,
        )
        neg = pst.tile([P, M_TILES], FP32, tag="neg")
        nc.vector.tensor_sub(neg[:], rows[:], dex[:])
        lneg = pst.tile([P, M_TILES], FP32, tag="lneg")
        nc.scalar.activation(lneg[:], neg[:], AF.Ln)
        lp = pst.tile([P, M_TILES], FP32, tag="lp")
        nc.vector.tensor_sub(lp[:], lneg[:], st[:])
        nc.sync.dma_start(out_parts, lp[:])

    nc.compile()
    return nc


def get_nc():
    if "nc" not in _CACHE:
        _CACHE["nc"] = _build_nc()
    return _CACHE["nc"]


def make_in_maps(z1, z2):
    z1 = np.ascontiguousarray(np.asarray(z1, dtype=np.float32))
    z2 = np.ascontiguousarray(np.asarray(z2, dtype=np.float32))
    in_maps = []
    for c in range(N_CORES):
        blk = slice(c * M_LOC, (c + 1) * M_LOC)
        in_maps.append({"z1": z1[blk], "z2": z2, "z2d": z2[blk]})
    return in_maps


def kernel(z1, z2):
    from concourse.bass_utils import run_bass_kernel_spmd

    nc = get_nc()
    res = run_bass_kernel_spmd(nc, make_in_maps(z1, z2), core_ids=list(range(N_CORES)))
    total = 0.0
    for c in range(N_CORES):
        total += res.results[c]["loss_parts"].astype(np.float64).sum()
    return np.float32(total)
